# revision 11
# baseline (speedup 1.0000x reference)
"""DMPNN encoder on 8 trn2 NeuronCores (Bass/Tile), data-parallel over bonds/atoms/graphs.

Self-contained: only needs numpy/jax/concourse (present in the environment).

Strategy:
- Shard bonds (200000 -> 8 x 25000, padded to 25088), atoms (12500 -> 12544),
  graphs (512/core) contiguously; shard boundaries align with graph boundaries.
- message recurrence m_{t+1} = relu(gather_sum(m_t) @ W_h + pre) is rewritten as
  m_{t+1} = relu(gather_sum(z_t) + pre) with z_t = m_t @ W_h (linearity), so the
  random-index gather happens on z and the matmul needs only m^T (PE transpose).
- Each iteration: fused per-tile phase computes m (from gathered z of prev iter)
  and z (matmul), writes z shard; AllGather z shards into a full fp16 table
  (+ zero sentinel row for masked neighbor slots); next iteration gathers rows
  via indirect DMA (128 rows/instr).
- Readout: AllGather final m, gather a2b rows, atom_out = relu(concat @ W_o + b_o),
  per-graph mean via 0/1 S-matrix matmuls accumulated in PSUM, exact f32 1/count
  scaling on device.
"""
import numpy as np

# ---------------------------------------------------------------- constants
NB, NA, NG = 200000, 100000, 4096
MAX_NB, AF, BF, H, DEPTH = 6, 133, 14, 300, 8
C = 8
NBC, NAC, NGC = NB // C, NA // C, NG // C          # 25000, 12500, 512
NBP = ((NBC + 127) // 128) * 128                   # 25088
NAP = ((NAC + 127) // 128) * 128                   # 12544
NT_B = NBP // 128                                  # 196 bond tiles
NT_A = NAP // 128                                  # 98 atom tiles
HP = 320                                           # padded hidden
AFP, BFP = 144, 16
CF = AFP + HP                                      # 464
SENT = C * NBP                                     # zero-sentinel row id
HCH = [(0, 128), (128, 128), (256, 64)]            # hidden K/M chunks

_CACHE = {}
DEBUG = False


def _install_compat():
    import concourse.tile as tile
    from concourse.vector_clock import ScopedClock
    import bass_rust as br

    def patched_drain_and_barrier(self, tick_clock, wait_clock):
        probe = self.nc.sync.nop(nofuse=True, hint="pre_drain_waits")
        wait_clock.add_sem_waits(probe.ins, ScopedClock({None: tick_clock.global_clock}))
        si = probe.ins.sync_info
        if si is not None and si.on_wait and len(si.on_wait) > 1:
            waits = list(si.on_wait)
            upds = list(si.on_update or [])
            probe.ins.sync_info = br.SyncInfo(on_wait=[waits[0]], on_update=upds)
            for w in waits[1:]:
                n2 = self.nc.sync.nop(nofuse=True, hint="pre_drain_waits2")
                n2.ins.sync_info = br.SyncInfo(on_wait=[w], on_update=[])
        self.nc.sync.drain()
        self.nc.all_engine_barrier()
        assert self.sems is not None
        popped = self.nc._tile_sem_poison_stack.pop()
        assert popped is self._sem_poison
        self.nc.clear_and_free_semaphores(list(self.sems.allocated().values()))
        self.nc.all_engine_barrier()

    tile.TileContext._drain_and_barrier = patched_drain_and_barrier

    from concourse import bass_utils as bu
    if not getattr(bu, "_dge_patched", False):
        orig = bu.get_walrus_args

        def get_walrus_args_dge(arch, tmpdir, *, dve_root=None):
            return orig(arch, tmpdir, dve_root=dve_root) + [
                "--dge-levels=io,spill_reload,scalar_dynamic_offset,vector_dynamic_offsets,dst_reduce"
            ]

        bu.get_walrus_args = get_walrus_args_dge
        bu._dge_patched = True


def _split_sync_waits(nc, maxw=1):
    from concourse import mybir
    cnt = [0]
    for f in nc.m.functions:
        for bb in f.blocks:
            out = []
            for inst in bb.instructions:
                si = getattr(inst, "sync_info", None)
                on_wait = list(si.on_wait) if (si is not None and si.on_wait) else []
                if len(on_wait) > maxw:
                    for w in on_wait[:-maxw]:
                        cnt[0] += 1
                        nop = mybir.InstNoOp(
                            name=f"WS-{cnt[0]}-{inst.name}",
                            sync_info=mybir.SyncInfo(on_wait=[w], on_update=[]),
                            bass_nofuse=True,
                            engine=inst.engine,
                        )
                        out.append(nop)
                    inst.sync_info = mybir.SyncInfo(
                        on_wait=on_wait[-maxw:], on_update=list(si.on_update or [])
                    )
                out.append(inst)
            bb.instructions[:] = out


# ---------------------------------------------------------------- bass program
def _build_program():
    from concourse import bass, mybir
    import concourse.tile as tile
    from concourse.masks import make_identity

    FP16 = mybir.dt.float16
    F32 = mybir.dt.float32
    I32 = mybir.dt.int32

    nc = bass.Bass(num_devices=C)

    # inputs (per core)
    afull = nc.dram_tensor("afull", [NA, AFP], FP16, kind="ExternalInput")
    aslice = nc.dram_tensor("aslice", [NAP, AFP], FP16, kind="ExternalInput")
    bfp = nc.dram_tensor("bfp", [NBP, BFP], FP16, kind="ExternalInput")
    b2a_t = nc.dram_tensor("b2a_t", [NT_B, 128, 1], I32, kind="ExternalInput")
    idxm = nc.dram_tensor("idxm", [NT_B, 128, MAX_NB], I32, kind="ExternalInput")
    idxa = nc.dram_tensor("idxa", [NT_A, 128, MAX_NB], I32, kind="ExternalInput")
    smat = nc.dram_tensor("smat", [NT_A, 128, NGC], FP16, kind="ExternalInput")
    recip = nc.dram_tensor("recip", [NGC // 128, 128, 1], F32, kind="ExternalInput")
    wcati = nc.dram_tensor("wcati", [AFP + BFP, HP], FP16, kind="ExternalInput")
    whp = nc.dram_tensor("whp", [HP, HP], FP16, kind="ExternalInput")
    wcato = nc.dram_tensor("wcato", [CF, HP], FP16, kind="ExternalInput")
    bo = nc.dram_tensor("bo", [128, HP], FP16, kind="ExternalInput")

    # internal
    z_shard = nc.dram_tensor("z_shard", [NBP, HP], FP16)
    m_shard = nc.dram_tensor("m_shard", [NBP, HP], FP16)
    z_full = nc.dram_tensor("z_full", [C * NBP + 1, HP], FP16, addr_space="Shared")
    m_full = nc.dram_tensor("m_full", [C * NBP + 1, HP], FP16, addr_space="Shared")

    outc = nc.dram_tensor("outc", [NGC, H], F32, kind="ExternalOutput")
    if DEBUG:
        dbg_m1 = nc.dram_tensor("dbg_m1", [NBP, HP], FP16, kind="ExternalOutput")
        dbg_z = nc.dram_tensor("dbg_z", [NBP, HP], FP16, kind="ExternalOutput")
        dbg_zf = nc.dram_tensor("dbg_zf", [C * NBP + 1, HP], FP16, kind="ExternalOutput")
        dbg_pre = nc.dram_tensor("dbg_pre", [128, NT_B * HP], FP16, kind="ExternalOutput")
        dbg_mf = nc.dram_tensor("dbg_mf", [NBP, HP], FP16, kind="ExternalOutput")
        dbg_ao = nc.dram_tensor("dbg_ao", [NAP, HP], FP16, kind="ExternalOutput")
        dbg_ah = nc.dram_tensor("dbg_ah", [NAP, HP], FP16, kind="ExternalOutput")
        dbg_pT = nc.dram_tensor("dbg_pT", [128, 3, NGC], F32, kind="ExternalOutput")

    GRP = 2  # bond tiles per group for DVE batching

    with tile.TileContext(nc) as tc:
        with (
            tc.tile_pool(name="const", bufs=1) as const,
            tc.tile_pool(name="resident", bufs=1) as res,
            tc.tile_pool(name="work", bufs=3) as work,
            tc.tile_pool(name="gath", bufs=2) as gath,
            tc.tile_pool(name="ps_t", bufs=2, space="PSUM") as ps_t,
            tc.tile_pool(name="ps_mm", bufs=2, space="PSUM") as ps_mm,
            tc.tile_pool(name="ps_pool", bufs=1, space="PSUM") as ps_pool,
        ):
            ident16 = const.tile([128, 128], FP16)
            make_identity(nc, ident16[:])
            ident32 = const.tile([128, 128], F32)
            make_identity(nc, ident32[:])

            # resident weights, one SBUF tile per K-chunk (<=128 partitions each)
            def load_chunks(dram, chunks, name):
                tiles = []
                for i, (lo, K) in enumerate(chunks):
                    t = const.tile([K, HP], FP16, tag=f"{name}{i}")
                    nc.sync.dma_start(out=t[:], in_=dram[lo : lo + K])
                    tiles.append(t)
                return tiles

            ICH = ((0, 128), (128, 16), (144, 16))
            OCH = ((0, 128), (128, 16), (144, 128), (272, 128), (400, 64))
            wcati_sb = load_chunks(wcati, ICH, "wi")
            whp_sb = load_chunks(whp, HCH, "wh")
            wcato_sb = load_chunks(wcato, OCH, "wo")
            bo_sb = const.tile([128, HP], FP16)
            nc.sync.dma_start(out=bo_sb[:], in_=bo[:])

            # zero sentinel rows
            zrow0 = const.tile([1, HP], FP16)
            nc.gpsimd.memset(zrow0[:], 0.0)
            nc.sync.dma_start(out=z_full[C * NBP : C * NBP + 1], in_=zrow0[:])
            nc.sync.dma_start(out=m_full[C * NBP : C * NBP + 1], in_=zrow0[:])

            # resident pre-activation [128, NT_B*HP] fp16 (bond t*128+p -> [p, t*HP:...])
            pre_res = res.tile([128, NT_B * HP], FP16)

            def transpose_to(dst_view, src_view, fp16=True):
                """PE-transpose src [128, k] -> dst [k, 128] (k<=128)."""
                k = src_view.shape[-1]
                ps = ps_t.tile([128, 128], FP16 if fp16 else F32, tag="trps")
                nc.tensor.transpose(
                    out=ps[:k, :],
                    in_=src_view,
                    identity=(ident16 if fp16 else ident32)[:],
                )
                nc.vector.tensor_copy(out=dst_view, in_=ps[:k, :])

            # ---------------- stage 1: pre = concat(atom[b2a], bond) @ W_i; m1 = relu(pre)
            for t in range(NT_B):
                bt = work.tile([128, 1], I32, tag="b2a")
                nc.sync.dma_start(out=bt[:], in_=b2a_t[t])
                af = gath.tile([128, AFP], FP16, tag="af")
                nc.gpsimd.indirect_dma_start(
                    out=af[:], out_offset=None, in_=afull[:],
                    in_offset=bass.IndirectOffsetOnAxis(ap=bt[:], axis=0),
                )
                bf = work.tile([128, BFP], FP16, tag="bf")
                nc.sync.dma_start(out=bf[:], in_=bfp[t * 128 : (t + 1) * 128])
                cT = work.tile([128, 3, 128], FP16, tag="cT1")
                transpose_to(cT[:, 0, :], af[:, 0:128])
                transpose_to(cT[:16, 1, :], af[:, 128:144])
                transpose_to(cT[:16, 2, :], bf[:, 0:16])
                pre_ps = ps_mm.tile([128, HP], F32, tag="mm")
                for k, (lo, K) in enumerate(ICH):
                    nc.tensor.matmul(
                        out=pre_ps[:], lhsT=cT[:K, k, :], rhs=wcati_sb[k][:],
                        start=(k == 0), stop=(k == 2),
                    )
                nc.vector.tensor_copy(
                    out=pre_res[:, t * HP : (t + 1) * HP], in_=pre_ps[:]
                )
                m1 = work.tile([128, HP], FP16, tag="m1")
                nc.scalar.activation(
                    out=m1[:], in_=pre_ps[:], func=mybir.ActivationFunctionType.Relu
                )
                nc.sync.dma_start(out=m_shard[t * 128 : (t + 1) * 128], in_=m1[:])

            # helper: z tiles from an m group tile [128, GRP, HP] -> z_shard rows
            def z_from_m(mn, g):
                mT = work.tile([128, 3, GRP * 128], FP16, tag="mT")
                for j in range(GRP):
                    for k, (lo, K) in enumerate(HCH):
                        transpose_to(
                            mT[:K, k, j * 128 : (j + 1) * 128],
                            mn[:, j, lo : lo + K],
                        )
                zr = work.tile([128, GRP, HP], FP16, tag="zrow")
                for j in range(GRP):
                    z_ps = ps_mm.tile([128, HP], F32, tag="mm")
                    for k, (lo, K) in enumerate(HCH):
                        nc.tensor.matmul(
                            out=z_ps[:],
                            lhsT=mT[:K, k, j * 128 : (j + 1) * 128],
                            rhs=whp_sb[k][:],
                            start=(k == 0), stop=(k == 2),
                        )
                    nc.vector.tensor_copy(out=zr[:, j, :], in_=z_ps[:])
                nc.sync.dma_start(
                    out=z_shard[g * GRP * 128 : (g + 1) * GRP * 128].rearrange(
                        "(j p) h -> p j h", p=128
                    ),
                    in_=zr[:],
                )

            if DEBUG:
                nc.sync.dma_start(out=dbg_m1[:], in_=m_shard[:])
                nc.sync.dma_start(out=dbg_pre[:], in_=pre_res[:])
            NGRP = NT_B // GRP
            # first z from m1: stream m_shard back (keeps stage1 simple)
            for g in range(NGRP):
                mn = work.tile([128, GRP, HP], FP16, tag="mload")
                nc.sync.dma_start(
                    out=mn[:],
                    in_=m_shard[g * GRP * 128 : (g + 1) * GRP * 128].rearrange(
                        "(j p) h -> p j h", p=128
                    ),
                )
                z_from_m(mn, g)

            if DEBUG:
                nc.sync.dma_start(out=dbg_z[:], in_=z_shard[:])
            # ---------------- message-passing iterations
            for it in range(1, DEPTH):
                last = it == DEPTH - 1
                nc.gpsimd.collective_compute(
                    "AllGather", mybir.AluOpType.bypass,
                    replica_groups=[list(range(C))],
                    ins=[z_shard[:]], outs=[z_full[0 : C * NBP]],
                )
                if DEBUG and it == 1:
                    nc.sync.dma_start(out=dbg_zf[:], in_=z_full[:])
                for g in range(NGRP):
                    ix = work.tile([128, GRP, MAX_NB], I32, tag="idx")
                    nc.sync.dma_start(
                        out=ix[:],
                        in_=idxm[g * GRP : (g + 1) * GRP].rearrange("j p n -> p j n"),
                    )
                    gts = []
                    for n in range(MAX_NB):
                        gt = gath.tile([128, GRP, HP], FP16, tag=f"g{n}")
                        for j in range(GRP):
                            nc.gpsimd.indirect_dma_start(
                                out=gt[:, j, :], out_offset=None, in_=z_full[:],
                                in_offset=bass.IndirectOffsetOnAxis(
                                    ap=ix[:, j, n : n + 1], axis=0
                                ),
                            )
                        gts.append(gt)
                    u = work.tile([128, GRP, HP], FP16, tag="u")
                    nc.vector.tensor_tensor(
                        out=u[:], in0=gts[0][:], in1=gts[1][:], op=mybir.AluOpType.add
                    )
                    for n in range(2, MAX_NB):
                        nc.vector.tensor_tensor(
                            out=u[:], in0=u[:], in1=gts[n][:], op=mybir.AluOpType.add
                        )
                    nc.vector.tensor_tensor(
                        out=u[:], in0=u[:],
                        in1=pre_res[:, g * GRP * HP : (g + 1) * GRP * HP].rearrange(
                            "p (j h) -> p j h", h=HP
                        ),
                        op=mybir.AluOpType.add,
                    )
                    mn = work.tile([128, GRP, HP], FP16, tag="mn")
                    nc.scalar.activation(
                        out=mn[:], in_=u[:], func=mybir.ActivationFunctionType.Relu
                    )
                    if last:
                        nc.sync.dma_start(
                            out=m_shard[g * GRP * 128 : (g + 1) * GRP * 128].rearrange(
                                "(j p) h -> p j h", p=128
                            ),
                            in_=mn[:],
                        )
                    else:
                        z_from_m(mn, g)

            if DEBUG:
                nc.sync.dma_start(out=dbg_mf[:], in_=m_shard[:])
            # ---------------- readout
            nc.gpsimd.collective_compute(
                "AllGather", mybir.AluOpType.bypass,
                replica_groups=[list(range(C))],
                ins=[m_shard[:]], outs=[m_full[0 : C * NBP]],
            )
            pool_acc = res.tile([128, 3, NGC], F32)
            nc.vector.memset(pool_acc[:], 0.0)
            for t in range(NT_A):
                ix = work.tile([128, MAX_NB], I32, tag="idxa")
                nc.sync.dma_start(out=ix[:], in_=idxa[t])
                gts = []
                for n in range(MAX_NB):
                    gt = gath.tile([128, HP], FP16, tag=f"ga{n}")
                    nc.gpsimd.indirect_dma_start(
                        out=gt[:], out_offset=None, in_=m_full[:],
                        in_offset=bass.IndirectOffsetOnAxis(ap=ix[:, n : n + 1], axis=0),
                    )
                    gts.append(gt)
                ah = work.tile([128, HP], FP16, tag="ah")
                nc.vector.tensor_tensor(
                    out=ah[:], in0=gts[0][:], in1=gts[1][:], op=mybir.AluOpType.add
                )
                for n in range(2, MAX_NB):
                    nc.vector.tensor_tensor(
                        out=ah[:], in0=ah[:], in1=gts[n][:], op=mybir.AluOpType.add
                    )
                af = work.tile([128, AFP], FP16, tag="afr")
                nc.sync.dma_start(out=af[:], in_=aslice[t * 128 : (t + 1) * 128])
                cT = work.tile([128, 5, 128], FP16, tag="cTo")
                transpose_to(cT[:, 0, :], af[:, 0:128])
                transpose_to(cT[:16, 1, :], af[:, 128:144])
                transpose_to(cT[:, 2, :], ah[:, 0:128])
                transpose_to(cT[:, 3, :], ah[:, 128:256])
                transpose_to(cT[:64, 4, :], ah[:, 256:320])
                ao_ps = ps_mm.tile([128, HP], F32, tag="mm")
                for k, (lo, K) in enumerate(OCH):
                    nc.tensor.matmul(
                        out=ao_ps[:], lhsT=cT[:K, k, :], rhs=wcato_sb[k][:],
                        start=(k == 0), stop=(k == 4),
                    )
                aosum = work.tile([128, HP], F32, tag="aosum")
                nc.vector.tensor_tensor(
                    out=aosum[:], in0=ao_ps[:], in1=bo_sb[:], op=mybir.AluOpType.add
                )
                ao = work.tile([128, HP], FP16, tag="ao")
                nc.scalar.activation(
                    out=ao[:], in_=aosum[:], func=mybir.ActivationFunctionType.Relu
                )
                if DEBUG:
                    nc.sync.dma_start(out=dbg_ao[t * 128 : (t + 1) * 128], in_=ao[:])
                    nc.sync.dma_start(out=dbg_ah[t * 128 : (t + 1) * 128], in_=ah[:])
                st = work.tile([128, NGC], FP16, tag="smat")
                nc.sync.dma_start(out=st[:], in_=smat[t])
                pk = ps_pool.tile([128, 3, NGC], F32)
                for k, (lo, K) in enumerate(HCH):
                    nc.tensor.matmul(
                        out=pk[:K, k, :], lhsT=ao[:, lo : lo + K], rhs=st[:],
                        start=True, stop=True,
                    )
                nc.vector.tensor_tensor(
                    out=pool_acc[:], in0=pool_acc[:], in1=pk[:], op=mybir.AluOpType.add
                )
            # finalize: transpose pooled [h,g] -> [g,h], scale by 1/count, store
            pooledT = pool_acc
            if DEBUG:
                nc.sync.dma_start(out=dbg_pT[:], in_=pooledT[:])
            for gt in range(NGC // 128):
                rc = work.tile([128, 1], F32, tag="recip")
                nc.sync.dma_start(out=rc[:], in_=recip[gt])
                og = work.tile([128, HP], F32, tag="og")
                for k, (lo, K) in enumerate(HCH):
                    ps = ps_t.tile([128, 128], F32, tag="trps")
                    nc.tensor.transpose(
                        out=ps[:, :K],
                        in_=pooledT[:K, k, gt * 128 : (gt + 1) * 128],
                        identity=ident32[:K, :K],
                    )
                    nc.vector.tensor_copy(out=og[:, lo : lo + K], in_=ps[:, :K])
                nc.vector.tensor_scalar(
                    out=og[:], in0=og[:], scalar1=rc[:], scalar2=None,
                    op0=mybir.AluOpType.mult,
                )
                nc.sync.dma_start(
                    out=outc[gt * 128 : (gt + 1) * 128], in_=og[:, 0:H]
                )

    _split_sync_waits(nc, maxw=1)
    return nc


# ---------------------------------------------------------------- host prep
def _pid(g):
    """global bond id -> padded table row id"""
    return (g // NBC) * NBP + (g % NBC)


def _prep_inputs(atom_feats, bond_feats, a2b, b2a, b2revb, batch, W_i, W_h, W_o, b_o):
    f16 = np.float16
    afull = np.zeros((NA, AFP), f16)
    afull[:, :AF] = atom_feats.astype(f16)

    wcati = np.zeros((AFP + BFP, HP), f16)
    wcati[0:AF, :H] = W_i[0:AF].astype(f16)
    wcati[AFP : AFP + BF, :H] = W_i[AF : AF + BF].astype(f16)
    whp = np.zeros((HP, HP), f16)
    whp[:H, :H] = W_h.astype(f16)
    wcato = np.zeros((CF, HP), f16)
    wcato[0:AF, :H] = W_o[0:AF].astype(f16)
    wcato[AFP : AFP + H, :H] = W_o[AF : AF + H].astype(f16)
    bo = np.zeros((128, HP), f16)
    bo[:, :H] = b_o.astype(f16)[None, :]

    a2b = np.asarray(a2b)
    b2a = np.asarray(b2a)
    b2revb = np.asarray(b2revb)
    batch = np.asarray(batch)

    counts = np.bincount(batch, minlength=NG).astype(np.float32)
    recip_all = (1.0 / np.maximum(counts, 1.0)).astype(np.float32)

    in_maps = []
    for c in range(C):
        bsl = slice(c * NBC, (c + 1) * NBC)
        asl = slice(c * NAC, (c + 1) * NAC)

        bfp = np.zeros((NBP, BFP), f16)
        bfp[:NBC, :BF] = bond_feats[bsl].astype(f16)

        aslice = np.zeros((NAP, AFP), f16)
        aslice[:NAC, :AF] = atom_feats[asl].astype(f16)

        b2a_c = np.zeros((NBP,), np.int32)
        b2a_c[:NBC] = b2a[bsl].astype(np.int32)
        b2a_t = b2a_c.reshape(NT_B, 128, 1)

        # message-passing neighbor table (into padded z/m table, SENT for masked)
        bonds_i = a2b[b2a[bsl]]                      # [NBC, 6] global bond ids
        valid = (bonds_i >= 0) & (bonds_i != b2revb[bsl][:, None])
        idx_c = np.where(valid, _pid(np.clip(bonds_i, 0, None)), SENT).astype(np.int32)
        idxm = np.full((NBP, MAX_NB), SENT, np.int32)
        idxm[:NBC] = idx_c
        idxm = idxm.reshape(NT_B, 128, MAX_NB)

        # readout a2b table
        rows = a2b[asl]
        validr = rows >= 0
        idxa_c = np.where(validr, _pid(np.clip(rows, 0, None)), SENT).astype(np.int32)
        idxa = np.full((NAP, MAX_NB), SENT, np.int32)
        idxa[:NAC] = idxa_c
        idxa = idxa.reshape(NT_A, 128, MAX_NB)

        # graph-pool selection matrix (0/1), graphs local to core
        smat = np.zeros((NAP, NGC), f16)
        gl = batch[asl].astype(np.int64) - c * NGC
        smat[np.arange(NAC), gl] = 1.0
        smat = smat.reshape(NT_A, 128, NGC)

        recip = recip_all[c * NGC : (c + 1) * NGC].reshape(NGC // 128, 128, 1)

        in_maps.append({
            "afull": afull, "aslice": aslice, "bfp": bfp, "b2a_t": b2a_t,
            "idxm": idxm, "idxa": idxa, "smat": smat, "recip": recip,
            "wcati": wcati, "whp": whp, "wcato": wcato, "bo": bo,
        })
    return in_maps


# ---------------------------------------------------------------- runner
class _SpmdRunner:
    def __init__(self, nc, n_cores=C):
        import jax
        from jax.sharding import Mesh, PartitionSpec
        from jax.experimental.shard_map import shard_map
        from concourse import mybir
        from concourse.bass2jax import (
            _bass_exec_p, install_neuronx_cc_hook, partition_id_tensor,
        )

        install_neuronx_cc_hook()
        self.jax = jax
        self.n_cores = n_cores
        partition_name = nc.partition_id_tensor.name if nc.partition_id_tensor else None
        in_names, out_names, out_avals, zero_outs = [], [], [], []
        for alloc in nc.m.functions[0].allocations:
            if not isinstance(alloc, mybir.MemoryLocationSet):
                continue
            name = alloc.memorylocations[0].name
            if alloc.kind == "ExternalInput":
                if name != partition_name:
                    in_names.append(name)
            elif alloc.kind == "ExternalOutput":
                out_names.append(name)
                shape = tuple(alloc.tensor_shape)
                dtype = mybir.dt.np(alloc.dtype)
                out_avals.append(jax.core.ShapedArray(shape, dtype))
                zero_outs.append(np.zeros(shape, dtype))
        self.in_names, self.out_names = in_names, out_names
        self.out_avals, self.zero_outs = out_avals, zero_outs
        n_params = len(in_names)
        all_in_names = in_names + out_names
        if partition_name is not None:
            all_in_names.append(partition_name)

        def _body(*args):
            operands = list(args)
            if partition_name is not None:
                operands.append(partition_id_tensor())
            outs = _bass_exec_p.bind(
                *operands,
                out_avals=tuple(out_avals),
                in_names=tuple(all_in_names),
                out_names=tuple(out_names),
                lowering_input_output_aliases=(),
                sim_require_finite=True,
                sim_require_nnan=True,
                nc=nc,
            )
            return tuple(outs)

        devices = jax.devices()[:n_cores]
        mesh = Mesh(np.asarray(devices), ("core",))
        n_outs = len(out_names)
        in_specs = (PartitionSpec("core"),) * (n_params + n_outs)
        out_specs = (PartitionSpec("core"),) * n_outs
        self.fn = jax.jit(
            shard_map(_body, mesh=mesh, in_specs=in_specs, out_specs=out_specs,
                      check_rep=False),
            keep_unused=True,
        )
        self.sharding = jax.sharding.NamedSharding(mesh, PartitionSpec("core"))

    def put_inputs(self, in_maps):
        concat_in = [
            np.concatenate([np.asarray(in_maps[c][n]) for c in range(self.n_cores)], axis=0)
            for n in self.in_names
        ]
        concat_zero = [
            np.zeros((self.n_cores * z.shape[0], *z.shape[1:]), z.dtype)
            for z in self.zero_outs
        ]
        return [self.jax.device_put(a, self.sharding) for a in concat_in + concat_zero]

    def run(self, device_args):
        outs = self.fn(*device_args)
        self.jax.block_until_ready(outs)
        return outs

    def results(self, outs):
        res = []
        for c in range(self.n_cores):
            d = {}
            for i, name in enumerate(self.out_names):
                d[name] = np.asarray(outs[i]).reshape(
                    self.n_cores, *self.out_avals[i].shape
                )[c]
            res.append(d)
        return res


def _get_runner():
    if "runner" not in _CACHE:
        _install_compat()
        nc = _build_program()
        _CACHE["runner"] = _SpmdRunner(nc)
    return _CACHE["runner"]


def kernel(atom_feats, bond_feats, a2b, b2a, b2revb, batch, W_i, W_h, W_o, b_o):
    runner = _get_runner()
    in_maps = _prep_inputs(
        np.asarray(atom_feats), np.asarray(bond_feats), np.asarray(a2b),
        np.asarray(b2a), np.asarray(b2revb), np.asarray(batch),
        np.asarray(W_i), np.asarray(W_h), np.asarray(W_o), np.asarray(b_o),
    )
    args = runner.put_inputs(in_maps)
    outs = runner.run(args)
    res = runner.results(outs)
    out = np.concatenate([res[c]["outc"] for c in range(C)], axis=0)
    return out.astype(np.float32)


# revision 13
# speedup vs baseline: 1.0673x; 1.0673x over previous
"""DMPNN encoder on 8 trn2 NeuronCores (Bass/Tile), data-parallel over bonds/atoms/graphs.

Self-contained: only needs numpy/jax/concourse (present in the environment).

Strategy:
- Shard bonds (200000 -> 8 x 25000, padded to 25088), atoms (12500 -> 12544),
  graphs (512/core) contiguously; shard boundaries align with graph boundaries.
- message recurrence m_{t+1} = relu(gather_sum(m_t) @ W_h + pre) is rewritten as
  m_{t+1} = relu(gather_sum(z_t) + pre) with z_t = m_t @ W_h (linearity), so the
  random-index gather happens on z and the matmul needs only m^T (PE transpose).
- Each iteration: fused per-tile phase computes m (from gathered z of prev iter)
  and z (matmul), writes z shard; AllGather z shards into a full fp16 table
  (+ zero sentinel row for masked neighbor slots); next iteration gathers rows
  via indirect DMA (128 rows/instr).
- Readout: AllGather final m, gather a2b rows, atom_out = relu(concat @ W_o + b_o),
  per-graph mean via 0/1 S-matrix matmuls accumulated in PSUM, exact f32 1/count
  scaling on device.
"""
import numpy as np

# ---------------------------------------------------------------- constants
NB, NA, NG = 200000, 100000, 4096
MAX_NB, AF, BF, H, DEPTH = 6, 133, 14, 300, 8
C = 8
NBC, NAC, NGC = NB // C, NA // C, NG // C          # 25000, 12500, 512
NBP = ((NBC + 127) // 128) * 128                   # 25088
NAP = ((NAC + 127) // 128) * 128                   # 12544
NT_B = NBP // 128                                  # 196 bond tiles
NT_A = NAP // 128                                  # 98 atom tiles
HP = 320                                           # padded hidden
AFP, BFP = 144, 16
CF = AFP + HP                                      # 464
QS = 4                                             # AllGather splits per iteration
NQR = NBP // QS                                    # rows per split (6272)
SENT = C * NBP                                     # zero-sentinel row id
HCH = [(0, 128), (128, 128), (256, 64)]            # hidden K/M chunks

_CACHE = {}
DEBUG = False


def _install_compat():
    import concourse.tile as tile
    from concourse.vector_clock import ScopedClock
    import bass_rust as br

    def patched_drain_and_barrier(self, tick_clock, wait_clock):
        probe = self.nc.sync.nop(nofuse=True, hint="pre_drain_waits")
        wait_clock.add_sem_waits(probe.ins, ScopedClock({None: tick_clock.global_clock}))
        si = probe.ins.sync_info
        if si is not None and si.on_wait and len(si.on_wait) > 1:
            waits = list(si.on_wait)
            upds = list(si.on_update or [])
            probe.ins.sync_info = br.SyncInfo(on_wait=[waits[0]], on_update=upds)
            for w in waits[1:]:
                n2 = self.nc.sync.nop(nofuse=True, hint="pre_drain_waits2")
                n2.ins.sync_info = br.SyncInfo(on_wait=[w], on_update=[])
        self.nc.sync.drain()
        self.nc.all_engine_barrier()
        assert self.sems is not None
        popped = self.nc._tile_sem_poison_stack.pop()
        assert popped is self._sem_poison
        self.nc.clear_and_free_semaphores(list(self.sems.allocated().values()))
        self.nc.all_engine_barrier()

    tile.TileContext._drain_and_barrier = patched_drain_and_barrier

    from concourse import bass_utils as bu
    if not getattr(bu, "_dge_patched", False):
        orig = bu.get_walrus_args

        def get_walrus_args_dge(arch, tmpdir, *, dve_root=None):
            return orig(arch, tmpdir, dve_root=dve_root) + [
                "--dge-levels=io,spill_reload,scalar_dynamic_offset,vector_dynamic_offsets,dst_reduce"
            ]

        bu.get_walrus_args = get_walrus_args_dge
        bu._dge_patched = True


def _split_sync_waits(nc, maxw=1):
    from concourse import mybir
    cnt = [0]
    for f in nc.m.functions:
        for bb in f.blocks:
            out = []
            for inst in bb.instructions:
                si = getattr(inst, "sync_info", None)
                on_wait = list(si.on_wait) if (si is not None and si.on_wait) else []
                if len(on_wait) > maxw:
                    for w in on_wait[:-maxw]:
                        cnt[0] += 1
                        nop = mybir.InstNoOp(
                            name=f"WS-{cnt[0]}-{inst.name}",
                            sync_info=mybir.SyncInfo(on_wait=[w], on_update=[]),
                            bass_nofuse=True,
                            engine=inst.engine,
                        )
                        out.append(nop)
                    inst.sync_info = mybir.SyncInfo(
                        on_wait=on_wait[-maxw:], on_update=list(si.on_update or [])
                    )
                out.append(inst)
            bb.instructions[:] = out


# ---------------------------------------------------------------- bass program
def _build_program():
    from concourse import bass, mybir
    import concourse.tile as tile
    from concourse.masks import make_identity

    FP16 = mybir.dt.float16
    F32 = mybir.dt.float32
    I32 = mybir.dt.int32

    nc = bass.Bass(num_devices=C)

    # inputs (per core)
    afull = nc.dram_tensor("afull", [NA, AFP], FP16, kind="ExternalInput")
    aslice = nc.dram_tensor("aslice", [NAP, AFP], FP16, kind="ExternalInput")
    bfp = nc.dram_tensor("bfp", [NBP, BFP], FP16, kind="ExternalInput")
    b2a_t = nc.dram_tensor("b2a_t", [NT_B, 128, 1], I32, kind="ExternalInput")
    idxm = nc.dram_tensor("idxm", [NT_B, 128, MAX_NB], I32, kind="ExternalInput")
    idxa = nc.dram_tensor("idxa", [NT_A, 128, MAX_NB], I32, kind="ExternalInput")
    smat = nc.dram_tensor("smat", [NT_A, 128, NGC], FP16, kind="ExternalInput")
    recip = nc.dram_tensor("recip", [NGC // 128, 128, 1], F32, kind="ExternalInput")
    wcati = nc.dram_tensor("wcati", [AFP + BFP, HP], FP16, kind="ExternalInput")
    whp = nc.dram_tensor("whp", [HP, HP], FP16, kind="ExternalInput")
    wcato = nc.dram_tensor("wcato", [CF, HP], FP16, kind="ExternalInput")
    bo = nc.dram_tensor("bo", [128, HP], FP16, kind="ExternalInput")

    # internal
    z_shard = nc.dram_tensor("z_shard", [NBP, HP], FP16)
    m_shard = nc.dram_tensor("m_shard", [NBP, HP], FP16)
    z_full = nc.dram_tensor("z_full", [C * NBP + 1, HP], FP16, addr_space="Shared")
    m_full = nc.dram_tensor("m_full", [C * NBP + 1, HP], FP16, addr_space="Shared")

    outc = nc.dram_tensor("outc", [NGC, H], F32, kind="ExternalOutput")
    if DEBUG:
        dbg_m1 = nc.dram_tensor("dbg_m1", [NBP, HP], FP16, kind="ExternalOutput")
        dbg_z = nc.dram_tensor("dbg_z", [NBP, HP], FP16, kind="ExternalOutput")
        dbg_zf = nc.dram_tensor("dbg_zf", [C * NBP + 1, HP], FP16, kind="ExternalOutput")
        dbg_pre = nc.dram_tensor("dbg_pre", [128, NT_B * HP], FP16, kind="ExternalOutput")
        dbg_mf = nc.dram_tensor("dbg_mf", [NBP, HP], FP16, kind="ExternalOutput")
        dbg_ao = nc.dram_tensor("dbg_ao", [NAP, HP], FP16, kind="ExternalOutput")
        dbg_ah = nc.dram_tensor("dbg_ah", [NAP, HP], FP16, kind="ExternalOutput")
        dbg_pT = nc.dram_tensor("dbg_pT", [128, 3, NGC], F32, kind="ExternalOutput")

    GRP = 2  # bond tiles per group for DVE batching

    with tile.TileContext(nc) as tc:
        with (
            tc.tile_pool(name="const", bufs=1) as const,
            tc.tile_pool(name="resident", bufs=1) as res,
            tc.tile_pool(name="work", bufs=3) as work,
            tc.tile_pool(name="gath", bufs=2) as gath,
            tc.tile_pool(name="ps_t", bufs=2, space="PSUM") as ps_t,
            tc.tile_pool(name="ps_mm", bufs=2, space="PSUM") as ps_mm,
            tc.tile_pool(name="ps_pool", bufs=1, space="PSUM") as ps_pool,
        ):
            ident16 = const.tile([128, 128], FP16)
            make_identity(nc, ident16[:])
            ident32 = const.tile([128, 128], F32)
            make_identity(nc, ident32[:])

            # resident weights, one SBUF tile per K-chunk (<=128 partitions each)
            def load_chunks(dram, chunks, name):
                tiles = []
                for i, (lo, K) in enumerate(chunks):
                    t = const.tile([K, HP], FP16, tag=f"{name}{i}")
                    nc.sync.dma_start(out=t[:], in_=dram[lo : lo + K])
                    tiles.append(t)
                return tiles

            ICH = ((0, 128), (128, 16), (144, 16))
            OCH = ((0, 128), (128, 16), (144, 128), (272, 128), (400, 64))
            wcati_sb = load_chunks(wcati, ICH, "wi")
            whp_sb = load_chunks(whp, HCH, "wh")
            wcato_sb = load_chunks(wcato, OCH, "wo")
            bo_sb = const.tile([128, HP], FP16)
            nc.sync.dma_start(out=bo_sb[:], in_=bo[:])

            # zero sentinel rows
            zrow0 = const.tile([1, HP], FP16)
            nc.gpsimd.memset(zrow0[:], 0.0)
            nc.sync.dma_start(out=z_full[C * NBP : C * NBP + 1], in_=zrow0[:])
            nc.sync.dma_start(out=m_full[C * NBP : C * NBP + 1], in_=zrow0[:])

            # resident pre-activation [128, NT_B*HP] fp16 (bond t*128+p -> [p, t*HP:...])
            pre_res = res.tile([128, NT_B * HP], FP16)

            def transpose_to(dst_view, src_view, fp16=True):
                """PE-transpose src [128, k] -> dst [k, 128] (k<=128)."""
                k = src_view.shape[-1]
                ps = ps_t.tile([128, 128], FP16 if fp16 else F32, tag="trps")
                nc.tensor.transpose(
                    out=ps[:k, :],
                    in_=src_view,
                    identity=(ident16 if fp16 else ident32)[:],
                )
                nc.vector.tensor_copy(out=dst_view, in_=ps[:k, :])

            # ---------------- stage 1: pre = concat(atom[b2a], bond) @ W_i; m1 = relu(pre)
            for t in range(NT_B):
                bt = work.tile([128, 1], I32, tag="b2a")
                nc.sync.dma_start(out=bt[:], in_=b2a_t[t])
                af = gath.tile([128, AFP], FP16, tag="af")
                nc.gpsimd.indirect_dma_start(
                    out=af[:], out_offset=None, in_=afull[:],
                    in_offset=bass.IndirectOffsetOnAxis(ap=bt[:], axis=0),
                )
                bf = work.tile([128, BFP], FP16, tag="bf")
                nc.sync.dma_start(out=bf[:], in_=bfp[t * 128 : (t + 1) * 128])
                cT = work.tile([128, 3, 128], FP16, tag="cT1")
                transpose_to(cT[:, 0, :], af[:, 0:128])
                transpose_to(cT[:16, 1, :], af[:, 128:144])
                transpose_to(cT[:16, 2, :], bf[:, 0:16])
                pre_ps = ps_mm.tile([128, HP], F32, tag="mm")
                for k, (lo, K) in enumerate(ICH):
                    nc.tensor.matmul(
                        out=pre_ps[:], lhsT=cT[:K, k, :], rhs=wcati_sb[k][:],
                        start=(k == 0), stop=(k == 2),
                    )
                nc.vector.tensor_copy(
                    out=pre_res[:, t * HP : (t + 1) * HP], in_=pre_ps[:]
                )
                m1 = work.tile([128, HP], FP16, tag="m1")
                nc.scalar.activation(
                    out=m1[:], in_=pre_ps[:], func=mybir.ActivationFunctionType.Relu
                )
                nc.sync.dma_start(out=m_shard[t * 128 : (t + 1) * 128], in_=m1[:])

            # helper: z tiles from an m group tile [128, GRP, HP] -> z_shard rows
            def z_from_m(mn, g):
                mT = work.tile([128, 3, GRP * 128], FP16, tag="mT")
                for j in range(GRP):
                    for k, (lo, K) in enumerate(HCH):
                        transpose_to(
                            mT[:K, k, j * 128 : (j + 1) * 128],
                            mn[:, j, lo : lo + K],
                        )
                zr = work.tile([128, GRP, HP], FP16, tag="zrow")
                for j in range(GRP):
                    z_ps = ps_mm.tile([128, HP], F32, tag="mm")
                    for k, (lo, K) in enumerate(HCH):
                        nc.tensor.matmul(
                            out=z_ps[:],
                            lhsT=mT[:K, k, j * 128 : (j + 1) * 128],
                            rhs=whp_sb[k][:],
                            start=(k == 0), stop=(k == 2),
                        )
                    nc.vector.tensor_copy(out=zr[:, j, :], in_=z_ps[:])
                nc.sync.dma_start(
                    out=z_shard[g * GRP * 128 : (g + 1) * GRP * 128].rearrange(
                        "(j p) h -> p j h", p=128
                    ),
                    in_=zr[:],
                )

            if DEBUG:
                nc.sync.dma_start(out=dbg_m1[:], in_=m_shard[:])
                nc.sync.dma_start(out=dbg_pre[:], in_=pre_res[:])
            NGRP = NT_B // GRP
            # first z from m1: stream m_shard back (keeps stage1 simple)
            for g in range(NGRP):
                mn = work.tile([128, GRP, HP], FP16, tag="mload")
                nc.sync.dma_start(
                    out=mn[:],
                    in_=m_shard[g * GRP * 128 : (g + 1) * GRP * 128].rearrange(
                        "(j p) h -> p j h", p=128
                    ),
                )
                z_from_m(mn, g)

            if DEBUG:
                nc.sync.dma_start(out=dbg_z[:], in_=z_shard[:])
            # ---------------- message-passing iterations
            for it in range(1, DEPTH):
                last = it == DEPTH - 1
                for q in range(QS):
                    nc.gpsimd.collective_compute(
                        "AllGather", mybir.AluOpType.bypass,
                        replica_groups=[list(range(C))],
                        ins=[z_shard[q * NQR : (q + 1) * NQR]],
                        outs=[z_full[q * C * NQR : (q + 1) * C * NQR]],
                    )
                if DEBUG and it == 1:
                    nc.sync.dma_start(out=dbg_zf[:], in_=z_full[:])
                for g in range(NGRP):
                    ix = work.tile([128, GRP, MAX_NB], I32, tag="idx")
                    nc.sync.dma_start(
                        out=ix[:],
                        in_=idxm[g * GRP : (g + 1) * GRP].rearrange("j p n -> p j n"),
                    )
                    gts = []
                    for n in range(MAX_NB):
                        gt = gath.tile([128, GRP, HP], FP16, tag=f"g{n}")
                        for j in range(GRP):
                            nc.gpsimd.indirect_dma_start(
                                out=gt[:, j, :], out_offset=None, in_=z_full[:],
                                in_offset=bass.IndirectOffsetOnAxis(
                                    ap=ix[:, j, n : n + 1], axis=0
                                ),
                            )
                        gts.append(gt)
                    u = work.tile([128, GRP, HP], FP16, tag="u")
                    nc.vector.tensor_tensor(
                        out=u[:], in0=gts[0][:], in1=gts[1][:], op=mybir.AluOpType.add
                    )
                    for n in range(2, MAX_NB):
                        nc.vector.tensor_tensor(
                            out=u[:], in0=u[:], in1=gts[n][:], op=mybir.AluOpType.add
                        )
                    nc.vector.tensor_tensor(
                        out=u[:], in0=u[:],
                        in1=pre_res[:, g * GRP * HP : (g + 1) * GRP * HP].rearrange(
                            "p (j h) -> p j h", h=HP
                        ),
                        op=mybir.AluOpType.add,
                    )
                    mn = work.tile([128, GRP, HP], FP16, tag="mn")
                    nc.scalar.activation(
                        out=mn[:], in_=u[:], func=mybir.ActivationFunctionType.Relu
                    )
                    if last:
                        nc.sync.dma_start(
                            out=m_shard[g * GRP * 128 : (g + 1) * GRP * 128].rearrange(
                                "(j p) h -> p j h", p=128
                            ),
                            in_=mn[:],
                        )
                    else:
                        z_from_m(mn, g)

            if DEBUG:
                nc.sync.dma_start(out=dbg_mf[:], in_=m_shard[:])
            # ---------------- readout
            for q in range(QS):
                nc.gpsimd.collective_compute(
                    "AllGather", mybir.AluOpType.bypass,
                    replica_groups=[list(range(C))],
                    ins=[m_shard[q * NQR : (q + 1) * NQR]],
                    outs=[m_full[q * C * NQR : (q + 1) * C * NQR]],
                )
            pool_acc = res.tile([128, 3, NGC], F32)
            nc.vector.memset(pool_acc[:], 0.0)
            for t in range(NT_A):
                ix = work.tile([128, MAX_NB], I32, tag="idxa")
                nc.sync.dma_start(out=ix[:], in_=idxa[t])
                gts = []
                for n in range(MAX_NB):
                    gt = gath.tile([128, HP], FP16, tag=f"ga{n}")
                    nc.gpsimd.indirect_dma_start(
                        out=gt[:], out_offset=None, in_=m_full[:],
                        in_offset=bass.IndirectOffsetOnAxis(ap=ix[:, n : n + 1], axis=0),
                    )
                    gts.append(gt)
                ah = work.tile([128, HP], FP16, tag="ah")
                nc.vector.tensor_tensor(
                    out=ah[:], in0=gts[0][:], in1=gts[1][:], op=mybir.AluOpType.add
                )
                for n in range(2, MAX_NB):
                    nc.vector.tensor_tensor(
                        out=ah[:], in0=ah[:], in1=gts[n][:], op=mybir.AluOpType.add
                    )
                af = work.tile([128, AFP], FP16, tag="afr")
                nc.sync.dma_start(out=af[:], in_=aslice[t * 128 : (t + 1) * 128])
                cT = work.tile([128, 5, 128], FP16, tag="cTo")
                transpose_to(cT[:, 0, :], af[:, 0:128])
                transpose_to(cT[:16, 1, :], af[:, 128:144])
                transpose_to(cT[:, 2, :], ah[:, 0:128])
                transpose_to(cT[:, 3, :], ah[:, 128:256])
                transpose_to(cT[:64, 4, :], ah[:, 256:320])
                ao_ps = ps_mm.tile([128, HP], F32, tag="mm")
                for k, (lo, K) in enumerate(OCH):
                    nc.tensor.matmul(
                        out=ao_ps[:], lhsT=cT[:K, k, :], rhs=wcato_sb[k][:],
                        start=(k == 0), stop=(k == 4),
                    )
                aosum = work.tile([128, HP], F32, tag="aosum")
                nc.vector.tensor_tensor(
                    out=aosum[:], in0=ao_ps[:], in1=bo_sb[:], op=mybir.AluOpType.add
                )
                ao = work.tile([128, HP], FP16, tag="ao")
                nc.scalar.activation(
                    out=ao[:], in_=aosum[:], func=mybir.ActivationFunctionType.Relu
                )
                if DEBUG:
                    nc.sync.dma_start(out=dbg_ao[t * 128 : (t + 1) * 128], in_=ao[:])
                    nc.sync.dma_start(out=dbg_ah[t * 128 : (t + 1) * 128], in_=ah[:])
                st = work.tile([128, NGC], FP16, tag="smat")
                nc.sync.dma_start(out=st[:], in_=smat[t])
                pk = ps_pool.tile([128, 3, NGC], F32)
                for k, (lo, K) in enumerate(HCH):
                    nc.tensor.matmul(
                        out=pk[:K, k, :], lhsT=ao[:, lo : lo + K], rhs=st[:],
                        start=True, stop=True,
                    )
                nc.vector.tensor_tensor(
                    out=pool_acc[:], in0=pool_acc[:], in1=pk[:], op=mybir.AluOpType.add
                )
            # finalize: transpose pooled [h,g] -> [g,h], scale by 1/count, store
            pooledT = pool_acc
            if DEBUG:
                nc.sync.dma_start(out=dbg_pT[:], in_=pooledT[:])
            for gt in range(NGC // 128):
                rc = work.tile([128, 1], F32, tag="recip")
                nc.sync.dma_start(out=rc[:], in_=recip[gt])
                og = work.tile([128, HP], F32, tag="og")
                for k, (lo, K) in enumerate(HCH):
                    ps = ps_t.tile([128, 128], F32, tag="trps")
                    nc.tensor.transpose(
                        out=ps[:, :K],
                        in_=pooledT[:K, k, gt * 128 : (gt + 1) * 128],
                        identity=ident32[:K, :K],
                    )
                    nc.vector.tensor_copy(out=og[:, lo : lo + K], in_=ps[:, :K])
                nc.vector.tensor_scalar(
                    out=og[:], in0=og[:], scalar1=rc[:], scalar2=None,
                    op0=mybir.AluOpType.mult,
                )
                nc.sync.dma_start(
                    out=outc[gt * 128 : (gt + 1) * 128], in_=og[:, 0:H]
                )

    _split_sync_waits(nc, maxw=1)
    return nc


# ---------------------------------------------------------------- host prep
def _pid(g):
    """global bond id -> padded table row id (quarter-blocked AllGather layout)"""
    c = g // NBC
    l = g % NBC
    q = l // NQR
    return q * (C * NQR) + c * NQR + (l % NQR)


def _prep_inputs(atom_feats, bond_feats, a2b, b2a, b2revb, batch, W_i, W_h, W_o, b_o):
    f16 = np.float16
    afull = np.zeros((NA, AFP), f16)
    afull[:, :AF] = atom_feats.astype(f16)

    wcati = np.zeros((AFP + BFP, HP), f16)
    wcati[0:AF, :H] = W_i[0:AF].astype(f16)
    wcati[AFP : AFP + BF, :H] = W_i[AF : AF + BF].astype(f16)
    whp = np.zeros((HP, HP), f16)
    whp[:H, :H] = W_h.astype(f16)
    wcato = np.zeros((CF, HP), f16)
    wcato[0:AF, :H] = W_o[0:AF].astype(f16)
    wcato[AFP : AFP + H, :H] = W_o[AF : AF + H].astype(f16)
    bo = np.zeros((128, HP), f16)
    bo[:, :H] = b_o.astype(f16)[None, :]

    a2b = np.asarray(a2b)
    b2a = np.asarray(b2a)
    b2revb = np.asarray(b2revb)
    batch = np.asarray(batch)

    counts = np.bincount(batch, minlength=NG).astype(np.float32)
    recip_all = (1.0 / np.maximum(counts, 1.0)).astype(np.float32)

    in_maps = []
    for c in range(C):
        bsl = slice(c * NBC, (c + 1) * NBC)
        asl = slice(c * NAC, (c + 1) * NAC)

        bfp = np.zeros((NBP, BFP), f16)
        bfp[:NBC, :BF] = bond_feats[bsl].astype(f16)

        aslice = np.zeros((NAP, AFP), f16)
        aslice[:NAC, :AF] = atom_feats[asl].astype(f16)

        b2a_c = np.zeros((NBP,), np.int32)
        b2a_c[:NBC] = b2a[bsl].astype(np.int32)
        b2a_t = b2a_c.reshape(NT_B, 128, 1)

        # message-passing neighbor table (into padded z/m table, SENT for masked)
        bonds_i = a2b[b2a[bsl]]                      # [NBC, 6] global bond ids
        valid = (bonds_i >= 0) & (bonds_i != b2revb[bsl][:, None])
        idx_c = np.where(valid, _pid(np.clip(bonds_i, 0, None)), SENT).astype(np.int32)
        idxm = np.full((NBP, MAX_NB), SENT, np.int32)
        idxm[:NBC] = idx_c
        idxm = idxm.reshape(NT_B, 128, MAX_NB)

        # readout a2b table
        rows = a2b[asl]
        validr = rows >= 0
        idxa_c = np.where(validr, _pid(np.clip(rows, 0, None)), SENT).astype(np.int32)
        idxa = np.full((NAP, MAX_NB), SENT, np.int32)
        idxa[:NAC] = idxa_c
        idxa = idxa.reshape(NT_A, 128, MAX_NB)

        # graph-pool selection matrix (0/1), graphs local to core
        smat = np.zeros((NAP, NGC), f16)
        gl = batch[asl].astype(np.int64) - c * NGC
        smat[np.arange(NAC), gl] = 1.0
        smat = smat.reshape(NT_A, 128, NGC)

        recip = recip_all[c * NGC : (c + 1) * NGC].reshape(NGC // 128, 128, 1)

        in_maps.append({
            "afull": afull, "aslice": aslice, "bfp": bfp, "b2a_t": b2a_t,
            "idxm": idxm, "idxa": idxa, "smat": smat, "recip": recip,
            "wcati": wcati, "whp": whp, "wcato": wcato, "bo": bo,
        })
    return in_maps


# ---------------------------------------------------------------- runner
class _SpmdRunner:
    def __init__(self, nc, n_cores=C):
        import jax
        from jax.sharding import Mesh, PartitionSpec
        from jax.experimental.shard_map import shard_map
        from concourse import mybir
        from concourse.bass2jax import (
            _bass_exec_p, install_neuronx_cc_hook, partition_id_tensor,
        )

        install_neuronx_cc_hook()
        self.jax = jax
        self.n_cores = n_cores
        partition_name = nc.partition_id_tensor.name if nc.partition_id_tensor else None
        in_names, out_names, out_avals, zero_outs = [], [], [], []
        for alloc in nc.m.functions[0].allocations:
            if not isinstance(alloc, mybir.MemoryLocationSet):
                continue
            name = alloc.memorylocations[0].name
            if alloc.kind == "ExternalInput":
                if name != partition_name:
                    in_names.append(name)
            elif alloc.kind == "ExternalOutput":
                out_names.append(name)
                shape = tuple(alloc.tensor_shape)
                dtype = mybir.dt.np(alloc.dtype)
                out_avals.append(jax.core.ShapedArray(shape, dtype))
                zero_outs.append(np.zeros(shape, dtype))
        self.in_names, self.out_names = in_names, out_names
        self.out_avals, self.zero_outs = out_avals, zero_outs
        n_params = len(in_names)
        all_in_names = in_names + out_names
        if partition_name is not None:
            all_in_names.append(partition_name)

        def _body(*args):
            operands = list(args)
            if partition_name is not None:
                operands.append(partition_id_tensor())
            outs = _bass_exec_p.bind(
                *operands,
                out_avals=tuple(out_avals),
                in_names=tuple(all_in_names),
                out_names=tuple(out_names),
                lowering_input_output_aliases=(),
                sim_require_finite=True,
                sim_require_nnan=True,
                nc=nc,
            )
            return tuple(outs)

        devices = jax.devices()[:n_cores]
        mesh = Mesh(np.asarray(devices), ("core",))
        n_outs = len(out_names)
        in_specs = (PartitionSpec("core"),) * (n_params + n_outs)
        out_specs = (PartitionSpec("core"),) * n_outs
        self.fn = jax.jit(
            shard_map(_body, mesh=mesh, in_specs=in_specs, out_specs=out_specs,
                      check_rep=False),
            keep_unused=True,
        )
        self.sharding = jax.sharding.NamedSharding(mesh, PartitionSpec("core"))

    def put_inputs(self, in_maps):
        concat_in = [
            np.concatenate([np.asarray(in_maps[c][n]) for c in range(self.n_cores)], axis=0)
            for n in self.in_names
        ]
        concat_zero = [
            np.zeros((self.n_cores * z.shape[0], *z.shape[1:]), z.dtype)
            for z in self.zero_outs
        ]
        return [self.jax.device_put(a, self.sharding) for a in concat_in + concat_zero]

    def run(self, device_args):
        outs = self.fn(*device_args)
        self.jax.block_until_ready(outs)
        return outs

    def results(self, outs):
        res = []
        for c in range(self.n_cores):
            d = {}
            for i, name in enumerate(self.out_names):
                d[name] = np.asarray(outs[i]).reshape(
                    self.n_cores, *self.out_avals[i].shape
                )[c]
            res.append(d)
        return res


def _get_runner():
    if "runner" not in _CACHE:
        _install_compat()
        nc = _build_program()
        _CACHE["runner"] = _SpmdRunner(nc)
    return _CACHE["runner"]


def kernel(atom_feats, bond_feats, a2b, b2a, b2revb, batch, W_i, W_h, W_o, b_o):
    runner = _get_runner()
    in_maps = _prep_inputs(
        np.asarray(atom_feats), np.asarray(bond_feats), np.asarray(a2b),
        np.asarray(b2a), np.asarray(b2revb), np.asarray(batch),
        np.asarray(W_i), np.asarray(W_h), np.asarray(W_o), np.asarray(b_o),
    )
    args = runner.put_inputs(in_maps)
    outs = runner.run(args)
    res = runner.results(outs)
    out = np.concatenate([res[c]["outc"] for c in range(C)], axis=0)
    return out.astype(np.float32)
